# revision 1
# baseline (speedup 1.0000x reference)
"""Trainium2 Bass kernel for nn_BaseGNNModel (2-layer GCN + image-query matmul).

Math (reference):
    norm = dinv[src] * w * dinv[dst],  dinv = rsqrt(segment_sum(w, dst))
    x1 = leaky_relu(segsum(norm * (NF @ W1)[src], dst) + b1, 0.2)
    x2 = segsum(norm * (x1 @ W2)[src], dst) + b2
    out = img @ x2.T                                  # [64, 20000]

Algebraic restructure used here (exact, up to fp reassociation):
    aggF = segsum(norm * NF[src], dst)                 # matmul commutes with segsum
    x1T  = leaky_relu(W1.T @ aggF.T + b1)              # [HID, N]
    PT   = W2 @ imgT                                   # [HID, B]
    QT   = x1T.T @ PT                                  # [N, B]
    out[:, n] = segsum(norm * QT[src], dst).T + img @ b2

Sharding: nodes (and their incoming edges) are range-sharded across the 8
cores; every segment-sum output is fully core-local.  Three SPMD launches:
  L1: per-core degree -> dinv, output replicated dinv table rows
  L2: layer-1 aggregation (gather + selection matmuls), x1T, PT, QT shard
  L3: final aggregation over QT -> output shard [64, N/8]
Host work between launches is pure layout (concat of shards); all arithmetic
runs on the NeuronCores.  Segment sums are computed as PE matmuls against
one-hot selection matrices built on-device with iota + is_equal, so duplicate
destinations accumulate exactly in fp32 PSUM.
"""

from contextlib import nullcontext

import numpy as np

from concourse import bacc, bass, mybir
from concourse.bass_utils import run_bass_kernel_spmd
from concourse.masks import make_identity
from concourse.tile import TileContext


def _maybe_reps(tc, reps):
    """Hardware repeat loop for timing (reps>1); no-op for production."""
    return tc.For_i(0, reps) if reps > 1 else nullcontext()

P = 128
NB = 125            # nodes per block (psum free dim)
FPAD = 320          # node-feature row padded to 320 f32 = 1280B (256B multiple)
F_TEXT = 300
NEG = 0.2

# full-size problem config
CFG_FULL = dict(B=64, N=20000, E=160000, HID=1024, OUT=1664, CORES=8)

TRACE = False                  # set by test.py for profiling
LAST_EXEC_NS = {}              # launch name -> exec ns (when TRACE)
LAST_BUILD = None              # (nc1, nc2, nc3) from the last kernel() call
LAST_MAPS = None               # {"l1": maps1, "l2": maps2, "l3": maps3}

_BUILD_CACHE = {}


# ----------------------------------------------------------------- host prep

def _prep_edges(edge_src, edge_dst, edge_weight, cfg):
    """Group edges by (core, block) of their dst; pad each block's edge list
    to T_b*128 where T_b is the max tile count for block index b across
    cores (SPMD needs identical program structure on every core)."""
    ncores = cfg["CORES"]
    npc = cfg["N"] // ncores               # nodes per core
    nblk = npc // NB                       # blocks per core
    assert npc % NB == 0

    core = edge_dst // npc
    blk = (edge_dst - core * npc) // NB
    dstl = (edge_dst - core * npc) - blk * NB

    # bucket edge ids per (core, block)
    buckets = [[None] * nblk for _ in range(ncores)]
    order = np.lexsort((blk, core))
    core_s, blk_s = core[order], blk[order]
    bounds = np.searchsorted(core_s * nblk + blk_s, np.arange(ncores * nblk + 1))
    for k in range(ncores):
        for b in range(nblk):
            i0, i1 = bounds[k * nblk + b], bounds[k * nblk + b + 1]
            buckets[k][b] = order[i0:i1]

    TBs = []
    for b in range(nblk):
        mx = max(len(buckets[k][b]) for k in range(ncores))
        TBs.append(max(1, -(-mx // P)))

    per_core = []
    for k in range(ncores):
        srcs, dls, ws = [], [], []
        for b in range(nblk):
            ids = buckets[k][b]
            pad = TBs[b] * P - len(ids)
            srcs.append(np.pad(edge_src[ids], (0, pad)))
            dls.append(np.pad(dstl[ids], (0, pad)))
            ws.append(np.pad(edge_weight[ids], (0, pad)))
        src = np.concatenate(srcs).astype(np.int64)
        dl = np.concatenate(dls).astype(np.float32)
        w = np.concatenate(ws).astype(np.float32)
        dst_glob = np.concatenate(
            [np.pad(edge_dst[buckets[k][b]], (0, TBs[b] * P - len(buckets[k][b])))
             for b in range(nblk)]).astype(np.int64)

        def idx16(a):
            # dma_gather layout: idx j at [j%16, j//16], replicated on 8
            # 16-partition groups -> [128, n/16]
            a16 = a.astype(np.int16).reshape(-1, 16).T
            return np.tile(a16, (8, 1)).copy()

        per_core.append(dict(
            src16=idx16(src),
            dst16=idx16(dst_glob),
            dstl=dl.reshape(-1, P).T.copy(),     # [128, TT]
            wts=w.reshape(-1, P).T.copy(),       # [128, TT]
        ))
    return TBs, per_core


# ------------------------------------------------------------------ builders

def _new_nc():
    return bacc.Bacc(None, target_bir_lowering=False)


def _iota_row(nc, pool):
    """[128, NB] f32 tile with value j in column j (every partition)."""
    ji = pool.tile([P, NB], mybir.dt.int32)
    nc.gpsimd.iota(ji[:], pattern=[[1, NB]], base=0, channel_multiplier=0)
    j = pool.tile([P, NB], mybir.dt.float32)
    nc.vector.tensor_copy(j[:], ji[:])
    return j


def _build_l1(TBs, cfg, reps=1):
    """deg -> dinv -> replicated dinv table rows [npc, 64]."""
    nc = _new_nc()
    nblk = len(TBs)
    TT = sum(TBs)
    npc = cfg["N"] // cfg["CORES"]

    dstl_in = nc.dram_tensor("dstl", [P, TT], mybir.dt.float32, kind="ExternalInput")
    wts_in = nc.dram_tensor("wts", [P, TT], mybir.dt.float32, kind="ExternalInput")
    drep_out = nc.dram_tensor("drep", [npc, 64], mybir.dt.float32, kind="ExternalOutput")

    with TileContext(nc) as tc:
        with (
            tc.tile_pool(name="sbA", bufs=1) as sbA,
            tc.tile_pool(name="sbS", bufs=4) as sbS,
            tc.tile_pool(name="sbR", bufs=2) as sbR,
            tc.tile_pool(name="ps", bufs=2, space="PSUM") as ps,
        ):
            with _maybe_reps(tc, reps):
                J = _iota_row(nc, sbA)
                dstl = sbA.tile([P, TT], mybir.dt.float32)
                wts = sbA.tile([P, TT], mybir.dt.float32)
                nc.sync.dma_start(out=dstl[:], in_=dstl_in[:])
                nc.sync.dma_start(out=wts[:], in_=wts_in[:])

                toff = 0
                for b in range(nblk):
                    Tb = TBs[b]
                    psd = ps.tile([NB, 1], mybir.dt.float32, space="PSUM", tag="deg")
                    for t in range(Tb):
                        col = toff + t
                        S0 = sbS.tile([P, NB], mybir.dt.float32, tag="S0")
                        nc.vector.tensor_scalar(
                            out=S0[:], in0=J[:],
                            scalar1=dstl[:, col:col + 1], scalar2=None,
                            op0=mybir.AluOpType.is_equal)
                        nc.tensor.matmul(out=psd[:], lhsT=S0[:], rhs=wts[:, col:col + 1],
                                         start=(t == 0), stop=(t == Tb - 1))
                    # dinv = sqrt(1/max(deg, deg==0 ? 1)) * (deg > 0)
                    m = sbS.tile([NB, 1], mybir.dt.float32, tag="m")
                    nc.vector.tensor_scalar(out=m[:], in0=psd[:], scalar1=0.0,
                                            scalar2=None, op0=mybir.AluOpType.is_gt)
                    safe = sbS.tile([NB, 1], mybir.dt.float32, tag="safe")
                    le = sbS.tile([NB, 1], mybir.dt.float32, tag="le")
                    nc.vector.tensor_scalar(out=le[:], in0=psd[:], scalar1=0.0,
                                            scalar2=None, op0=mybir.AluOpType.is_le)
                    nc.vector.tensor_tensor(out=safe[:], in0=psd[:], in1=le[:],
                                            op=mybir.AluOpType.add)
                    rec = sbS.tile([NB, 1], mybir.dt.float32, tag="rec")
                    nc.vector.reciprocal(rec[:], safe[:])
                    sq = sbS.tile([NB, 1], mybir.dt.float32, tag="sq")
                    nc.scalar.sqrt(sq[:], rec[:])
                    dv = sbS.tile([NB, 1], mybir.dt.float32, tag="dv")
                    nc.vector.tensor_tensor(out=dv[:], in0=sq[:], in1=m[:],
                                            op=mybir.AluOpType.mult)
                    rep = sbR.tile([NB, 64], mybir.dt.float32, tag="rep")
                    nc.vector.tensor_scalar(out=rep[:], in0=dv[:].to_broadcast([NB, 64]),
                                            scalar1=1.0, scalar2=None,
                                            op0=mybir.AluOpType.mult)
                    nc.sync.dma_start(out=drep_out[b * NB:(b + 1) * NB, :], in_=rep[:])
                    toff += Tb
    nc.finalize()
    return nc


def _build_l2(TBs, cfg, reps=1):
    nc = _new_nc()
    nblk = len(TBs)
    TT = sum(TBs)
    N, HID, OUT, B = cfg["N"], cfg["HID"], cfg["OUT"], cfg["B"]
    npc = N // cfg["CORES"]
    HCH = HID // P          # h chunks
    OCH = OUT // P          # o chunks
    NCW = 500 if npc % 500 == 0 else 250   # x1T n-chunk width
    NCH = npc // NCW

    nf_in = nc.dram_tensor("nf", [N, FPAD], mybir.dt.float32, kind="ExternalInput")
    drep_in = nc.dram_tensor("drep", [N, 64], mybir.dt.float32, kind="ExternalInput")
    w1_in = nc.dram_tensor("w1", [FPAD, HID], mybir.dt.float32r, kind="ExternalInput")
    w2_in = nc.dram_tensor("w2", [HID, OUT], mybir.dt.float32, kind="ExternalInput")
    b1_in = nc.dram_tensor("b1r", [1, HID], mybir.dt.float32r, kind="ExternalInput")
    img_in = nc.dram_tensor("img", [B, OUT], mybir.dt.float32, kind="ExternalInput")
    b2_in = nc.dram_tensor("b2r", [B, OUT], mybir.dt.float32, kind="ExternalInput")
    src_in = nc.dram_tensor("src16", [P, TT * 8], mybir.dt.int16, kind="ExternalInput")
    dsti_in = nc.dram_tensor("dst16", [P, TT * 8], mybir.dt.int16, kind="ExternalInput")
    dstl_in = nc.dram_tensor("dstl", [P, TT], mybir.dt.float32, kind="ExternalInput")
    wts_in = nc.dram_tensor("wts", [P, TT], mybir.dt.float32, kind="ExternalInput")

    qt_out = nc.dram_tensor("qt", [npc, 64], mybir.dt.float32, kind="ExternalOutput")
    c_out = nc.dram_tensor("cvec", [B, 1], mybir.dt.float32, kind="ExternalOutput")
    norm_out = nc.dram_tensor("normv", [P, TT], mybir.dt.float32, kind="ExternalOutput")

    with TileContext(nc) as tc:
        with (
            tc.tile_pool(name="sbA", bufs=1) as sbA,
            tc.tile_pool(name="sbG", bufs=3) as sbG,
            tc.tile_pool(name="sbD", bufs=2) as sbD,
            tc.tile_pool(name="sbS", bufs=4) as sbS,
            tc.tile_pool(name="sbW", bufs=2) as sbW,
            tc.tile_pool(name="ps1", bufs=1, space="PSUM") as ps1,
            tc.tile_pool(name="ps2", bufs=2, space="PSUM") as ps2,
        ):
            with _maybe_reps(tc, reps):
                J = _iota_row(nc, sbA)
                ident = sbA.tile([P, P], mybir.dt.float32)
                make_identity(nc, ident[:])

                # ---------- phase A: W2T / imgT / PT / cvec (overlaps gathers) --
                PTacc = sbA.tile([P, HCH * B], mybir.dt.float32)
                c_sb = sbA.tile([B, 1], mybir.dt.float32)
                with tc.tile_pool(name="sbE", bufs=1) as sbE:
                    imgs = sbE.tile([B, OUT], mybir.dt.float32)
                    nc.sync.dma_start(out=imgs[:], in_=img_in[:])
                    imgT = sbE.tile([P, OCH * B], mybir.dt.float32r)
                    for o in range(OCH):
                        tps = ps2.tile([P, B], mybir.dt.float32, space="PSUM", tag="tr")
                        nc.tensor.transpose(tps[:], imgs[:, o * P:(o + 1) * P],
                                            ident[:B, :B])
                        nc.vector.tensor_copy(imgT[:, o * B:(o + 1) * B], tps[:])
                    b2r = sbE.tile([B, OUT], mybir.dt.float32)
                    nc.sync.dma_start(out=b2r[:], in_=b2_in[:])
                    nc.vector.tensor_tensor(out=b2r[:], in0=imgs[:], in1=b2r[:],
                                            op=mybir.AluOpType.mult)
                    nc.vector.tensor_reduce(out=c_sb[:], in_=b2r[:],
                                            axis=mybir.AxisListType.X,
                                            op=mybir.AluOpType.add)
                    nc.sync.dma_start(out=c_out[:], in_=c_sb[:])

                    for o in range(OCH):
                        w2src = sbE.tile([P, HCH, P], mybir.dt.float32, tag="w2src",
                                         bufs=1)
                        nc.sync.dma_start(
                            out=w2src[:],
                            in_=bass.AP(w2_in, o * P,
                                        [[OUT, P], [OUT * P, HCH], [1, P]]))
                        w2T = sbE.tile([P, HID], mybir.dt.float32r, tag="w2T", bufs=1)
                        for h in range(HCH):
                            tps = ps2.tile([P, P], mybir.dt.float32, space="PSUM",
                                           tag="tr")
                            nc.tensor.transpose(tps[:], w2src[:, h, :], ident[:])
                            nc.vector.tensor_copy(w2T[:, h * P:(h + 1) * P], tps[:])
                        iT = imgT[:, o * B:(o + 1) * B]
                        pt_ps = ps1.tile([P, HCH * B], mybir.dt.float32, space="PSUM",
                                         tag="pt", name="pt_ps")
                        for h in range(HCH):
                            nc.tensor.matmul(out=pt_ps[:, h * B:(h + 1) * B],
                                             lhsT=w2T[:, h * P:(h + 1) * P], rhs=iT,
                                             start=True, stop=True)
                        if o == 0:
                            nc.vector.tensor_copy(PTacc[:], pt_ps[:])
                        else:
                            nc.vector.tensor_tensor(out=PTacc[:], in0=PTacc[:],
                                                    in1=pt_ps[:],
                                                    op=mybir.AluOpType.add)
                PT = sbA.tile([P, HCH * B], mybir.dt.bfloat16)
                nc.vector.tensor_copy(PT[:], PTacc[:])

                # ---------- phase B: gathers + layer-1 aggregation -------------
                src16 = sbA.tile([P, TT * 8], mybir.dt.int16)
                dst16 = sbA.tile([P, TT * 8], mybir.dt.int16)
                dstl = sbA.tile([P, TT], mybir.dt.float32)
                wts = sbA.tile([P, TT], mybir.dt.float32)
                nc.sync.dma_start(out=src16[:], in_=src_in[:])
                nc.sync.dma_start(out=dst16[:], in_=dsti_in[:])
                nc.sync.dma_start(out=dstl[:], in_=dstl_in[:])
                nc.sync.dma_start(out=wts[:], in_=wts_in[:])

                agg = [sbA.tile([P, npc], mybir.dt.float32r, tag=f"agg{i}",
                                name=f"agg{i}") for i in range(2)]
                agg.append(sbA.tile([64, npc], mybir.dt.float32r, tag="agg2",
                                    name="agg2"))
                norm_all = sbA.tile([P, TT], mybir.dt.float32)

                toff = 0
                for b in range(nblk):
                    Tb = TBs[b]
                    ni = Tb * P
                    # dinv gathers for this block's src and dst
                    dsg = sbD.tile([P, Tb, 64], mybir.dt.float32, tag="dsg")
                    ddg = sbD.tile([P, Tb, 64], mybir.dt.float32, tag="ddg")
                    nc.gpsimd.dma_gather(
                        out_ap=dsg[:], in_ap=drep_in[:],
                        idxs_ap=src16[:, toff * 8:(toff + Tb) * 8],
                        num_idxs=ni, num_idxs_reg=ni, elem_size=64, single_packet=False)
                    nc.gpsimd.dma_gather(
                        out_ap=ddg[:], in_ap=drep_in[:],
                        idxs_ap=dst16[:, toff * 8:(toff + Tb) * 8],
                        num_idxs=ni, num_idxs_reg=ni, elem_size=64, single_packet=False)
                    # norm = dinv_src * w * dinv_dst  [128, Tb]
                    nrm = norm_all[:, toff:toff + Tb]
                    nc.vector.tensor_tensor(
                        out=nrm, in0=dsg[:, :, 0],
                        in1=wts[:, toff:toff + Tb], op=mybir.AluOpType.mult)
                    nc.vector.tensor_tensor(
                        out=nrm, in0=nrm, in1=ddg[:, :, 0], op=mybir.AluOpType.mult)

                    # message gather + bf16 cast
                    mg = sbG.tile([P, Tb, FPAD], mybir.dt.float32, tag="mg")
                    nc.gpsimd.dma_gather(
                        out_ap=mg[:], in_ap=nf_in[:],
                        idxs_ap=src16[:, toff * 8:(toff + Tb) * 8],
                        num_idxs=ni, num_idxs_reg=ni, elem_size=FPAD, single_packet=False)
                    m16 = sbS.tile([P, Tb * FPAD], mybir.dt.bfloat16, tag="m16", bufs=2)
                    nc.scalar.activation(
                        out=m16[:], in_=mg[:].rearrange("p c d -> p (c d)"),
                        func=mybir.ActivationFunctionType.Copy)

                    pa = [ps1.tile([P, NB], mybir.dt.float32, space="PSUM", tag="pa0", name="pa0"),
                          ps1.tile([P, NB], mybir.dt.float32, space="PSUM", tag="pa1", name="pa1"),
                          ps1.tile([64, NB], mybir.dt.float32, space="PSUM", tag="pa2", name="pa2")]
                    for t in range(Tb):
                        col = toff + t
                        S0 = sbS.tile([P, NB], mybir.dt.bfloat16, tag="S0")
                        nc.vector.tensor_scalar(
                            out=S0[:], in0=J[:], scalar1=dstl[:, col:col + 1],
                            scalar2=None, op0=mybir.AluOpType.is_equal)
                        S1 = sbS.tile([P, NB], mybir.dt.bfloat16, tag="S1")
                        nc.vector.tensor_scalar(
                            out=S1[:], in0=S0[:], scalar1=norm_all[:, col:col + 1],
                            scalar2=None, op0=mybir.AluOpType.mult)
                        for fc, fw in ((0, P), (1, P), (2, 64)):
                            nc.tensor.matmul(
                                out=pa[fc][:, :],
                                lhsT=m16[:, t * FPAD + fc * P: t * FPAD + fc * P + fw],
                                rhs=S1[:],
                                start=(t == 0), stop=(t == Tb - 1))
                    for fc in range(3):
                        nc.vector.tensor_copy(agg[fc][:, b * NB:(b + 1) * NB], pa[fc][:])
                    toff += Tb
                nc.sync.dma_start(out=norm_out[:], in_=norm_all[:])

                # ---------- phase C: x1T = leaky(W1.T @ aggF + b1) --------------
                w1t = [sbA.tile([P, HID], mybir.dt.float32r, tag="w1k0", name="w1k0"),
                       sbA.tile([P, HID], mybir.dt.float32r, tag="w1k1", name="w1k1"),
                       sbA.tile([64, HID], mybir.dt.float32r, tag="w1k2", name="w1k2")]
                nc.sync.dma_start(out=w1t[0][:], in_=w1_in[0:P, :])
                nc.sync.dma_start(out=w1t[1][:], in_=w1_in[P:2 * P, :])
                nc.sync.dma_start(out=w1t[2][:], in_=w1_in[2 * P:2 * P + 64, :])
                b1row = sbA.tile([1, HID], mybir.dt.float32r)
                nc.sync.dma_start(out=b1row[:], in_=b1_in[:])
                ones_f = sbA.tile([1, npc], mybir.dt.float32)
                nc.vector.memset(ones_f[:], 1.0)
                ones = sbA.tile([1, npc], mybir.dt.float32r)
                nc.vector.tensor_copy(ones[:], ones_f[:])
                x1T = [sbA.tile([P, npc], mybir.dt.bfloat16, tag=f"x1T{h}",
                                name=f"x1T{h}") for h in range(HCH)]
                for h in range(HCH):
                    for nchi in range(NCH):
                        n0 = nchi * NCW
                        px = ps2.tile([P, NCW], mybir.dt.float32, space="PSUM", tag="xq")
                        for kc in range(3):
                            nc.tensor.matmul(
                                out=px[:],
                                lhsT=w1t[kc][:, h * P:(h + 1) * P],
                                rhs=agg[kc][:, n0:n0 + NCW],
                                start=(kc == 0), stop=False)
                        nc.tensor.matmul(
                            out=px[:], lhsT=b1row[:, h * P:(h + 1) * P],
                            rhs=ones[:, n0:n0 + NCW], start=False, stop=True)
                        t2 = sbS.tile([P, NCW], mybir.dt.float32, tag="t2", bufs=2)
                        nc.scalar.activation(out=t2[:], in_=px[:],
                                             func=mybir.ActivationFunctionType.Copy,
                                             scale=NEG)
                        nc.vector.tensor_tensor(out=x1T[h][:, n0:n0 + NCW],
                                                in0=px[:], in1=t2[:],
                                                op=mybir.AluOpType.max)

                # ---------- phase D: QT = x1T.T @ PT ---------------------------
                for nchi in range(npc // NB):
                    n0 = nchi * NB
                    pq = ps2.tile([NB, B], mybir.dt.float32, space="PSUM", tag="xq")
                    for h in range(HCH):
                        nc.tensor.matmul(
                            out=pq[:], lhsT=x1T[h][:, n0:n0 + NB],
                            rhs=PT[:, h * B:(h + 1) * B],
                            start=(h == 0), stop=(h == HCH - 1))
                    qsb = sbS.tile([NB, B], mybir.dt.float32, tag="qsb")
                    nc.vector.tensor_copy(qsb[:], pq[:])
                    nc.sync.dma_start(out=qt_out[n0:n0 + NB, :], in_=qsb[:])
    nc.finalize()
    return nc


def _build_l3(TBs, cfg, reps=1):
    nc = _new_nc()
    nblk = len(TBs)
    TT = sum(TBs)
    N, B = cfg["N"], cfg["B"]
    npc = N // cfg["CORES"]

    qt_in = nc.dram_tensor("qt", [N, 64], mybir.dt.float32, kind="ExternalInput")
    src_in = nc.dram_tensor("src16", [P, TT * 8], mybir.dt.int16, kind="ExternalInput")
    dstl_in = nc.dram_tensor("dstl", [P, TT], mybir.dt.float32, kind="ExternalInput")
    nrm_in = nc.dram_tensor("normv", [P, TT], mybir.dt.float32, kind="ExternalInput")
    c_in = nc.dram_tensor("cvec", [B, 1], mybir.dt.float32, kind="ExternalInput")
    out_own = nc.dram_tensor("outp", [B, npc], mybir.dt.float32, kind="ExternalOutput")

    with TileContext(nc) as tc:
        with (
            tc.tile_pool(name="sbA", bufs=1) as sbA,
            tc.tile_pool(name="sbG", bufs=3) as sbG,
            tc.tile_pool(name="sbS", bufs=4) as sbS,
            tc.tile_pool(name="ps", bufs=2, space="PSUM") as ps,
        ):
            with _maybe_reps(tc, reps):
                J = _iota_row(nc, sbA)
                src16 = sbA.tile([P, TT * 8], mybir.dt.int16)
                dstl = sbA.tile([P, TT], mybir.dt.float32)
                nrm = sbA.tile([P, TT], mybir.dt.float32)
                c_sb = sbA.tile([B, 1], mybir.dt.float32)
                nc.sync.dma_start(out=src16[:], in_=src_in[:])
                nc.sync.dma_start(out=dstl[:], in_=dstl_in[:])
                nc.sync.dma_start(out=nrm[:], in_=nrm_in[:])
                nc.sync.dma_start(out=c_sb[:], in_=c_in[:])

                toff = 0
                for b in range(nblk):
                    Tb = TBs[b]
                    ni = Tb * P
                    qg = sbG.tile([P, Tb, 64], mybir.dt.float32, tag="qg")
                    nc.gpsimd.dma_gather(
                        out_ap=qg[:], in_ap=qt_in[:],
                        idxs_ap=src16[:, toff * 8:(toff + Tb) * 8],
                        num_idxs=ni, num_idxs_reg=ni, elem_size=64, single_packet=False)
                    q16 = sbS.tile([P, Tb * 64], mybir.dt.bfloat16, tag="q16")
                    nc.scalar.activation(
                        out=q16[:], in_=qg[:].rearrange("p c d -> p (c d)"),
                        func=mybir.ActivationFunctionType.Copy)
                    po = ps.tile([B, NB], mybir.dt.float32, space="PSUM", tag="po")
                    for t in range(Tb):
                        col = toff + t
                        S0 = sbS.tile([P, NB], mybir.dt.bfloat16, tag="S0")
                        nc.vector.tensor_scalar(
                            out=S0[:], in0=J[:], scalar1=dstl[:, col:col + 1],
                            scalar2=None, op0=mybir.AluOpType.is_equal)
                        S1 = sbS.tile([P, NB], mybir.dt.bfloat16, tag="S1")
                        nc.vector.tensor_scalar(
                            out=S1[:], in0=S0[:], scalar1=nrm[:, col:col + 1],
                            scalar2=None, op0=mybir.AluOpType.mult)
                        nc.tensor.matmul(out=po[:],
                                         lhsT=q16[:, t * 64:t * 64 + B],
                                         rhs=S1[:],
                                         start=(t == 0), stop=(t == Tb - 1))
                    osb = sbS.tile([B, NB], mybir.dt.float32, tag="osb")
                    nc.vector.tensor_scalar(out=osb[:], in0=po[:], scalar1=c_sb[:, 0:1],
                                            scalar2=None, op0=mybir.AluOpType.add)
                    nc.sync.dma_start(out=out_own[:, b * NB:(b + 1) * NB], in_=osb[:])
                    toff += Tb
    nc.finalize()
    return nc


# ------------------------------------------------------------------- runner

def _run(name, nc, in_maps, cores):
    kw = {}
    if TRACE:
        kw = dict(trace=True)
    res = run_bass_kernel_spmd(nc, in_maps, core_ids=list(range(cores)), **kw)
    if res.exec_time_ns is not None:
        LAST_EXEC_NS[name] = res.exec_time_ns
    return res.results


def _kernel_impl(img_feat, node_features, edge_src, edge_dst, edge_weight,
                 W1, b1, W2, b2, cfg):
    ncores = cfg["CORES"]
    N, B, HID, OUT = cfg["N"], cfg["B"], cfg["HID"], cfg["OUT"]
    npc = N // ncores
    OCH = OUT // P

    TBs, per_core = _prep_edges(edge_src, edge_dst, edge_weight, cfg)
    key = (tuple(TBs), tuple(sorted(cfg.items())))
    if key not in _BUILD_CACHE:
        _BUILD_CACHE[key] = (_build_l1(TBs, cfg), _build_l2(TBs, cfg),
                             _build_l3(TBs, cfg))
    nc1, nc2, nc3 = _BUILD_CACHE[key]

    # ---- L1: dinv replicated table
    maps1 = [{"dstl": pc["dstl"], "wts": pc["wts"]} for pc in per_core]
    r1 = _run("l1", nc1, maps1, ncores)
    drep = np.concatenate([r1[k]["drep"] for k in range(ncores)], axis=0)

    # ---- L2
    nf_pad = np.zeros((N, FPAD), np.float32)
    nf_pad[:, :F_TEXT] = node_features
    w1_pad = np.zeros((FPAD, HID), np.float32)
    w1_pad[:F_TEXT, :] = W1
    b2rep = np.ascontiguousarray(np.broadcast_to(b2, (B, OUT))).astype(np.float32)
    maps2 = [dict(nf=nf_pad, drep=drep, w1=w1_pad, w2=W2, img=img_feat,
                  b1r=b1.reshape(1, -1).astype(np.float32), b2r=b2rep,
                  src16=pc["src16"], dst16=pc["dst16"],
                  dstl=pc["dstl"], wts=pc["wts"]) for pc in per_core]
    r2 = _run("l2", nc2, maps2, ncores)
    qt = np.concatenate([r2[k]["qt"] for k in range(ncores)], axis=0)
    cvec = r2[0]["cvec"]
    norms = [r2[k]["normv"] for k in range(ncores)]

    # ---- L3
    maps3 = [dict(qt=qt, src16=pc["src16"], dstl=pc["dstl"],
                  normv=norms[k], cvec=cvec)
             for k, pc in enumerate(per_core)]
    r3 = _run("l3", nc3, maps3, ncores)
    out = np.concatenate([r3[k]["outp"] for k in range(ncores)], axis=1)

    global LAST_BUILD, LAST_MAPS
    LAST_BUILD = (nc1, nc2, nc3)
    LAST_MAPS = {"l1": maps1, "l2": maps2, "l3": maps3}
    return out.astype(np.float32)


def kernel(img_feat, node_features, edge_src, edge_dst, edge_weight,
           W1, b1, W2, b2):
    return _kernel_impl(np.asarray(img_feat), np.asarray(node_features),
                        np.asarray(edge_src), np.asarray(edge_dst),
                        np.asarray(edge_weight), np.asarray(W1),
                        np.asarray(b1), np.asarray(W2), np.asarray(b2),
                        CFG_FULL)



# revision 3
# speedup vs baseline: 2.3374x; 2.3374x over previous
"""Trainium2 Bass kernel for nn_BaseGNNModel (2-layer GCN + image-query matmul).

Math (reference):
    norm = dinv[src] * w * dinv[dst],  dinv = rsqrt(segment_sum(w, dst))
    x1 = leaky_relu(segsum(norm * (NF @ W1)[src], dst) + b1, 0.2)
    x2 = segsum(norm * (x1 @ W2)[src], dst) + b2
    out = img @ x2.T                                  # [64, 20000]

Algebraic restructure used here (exact, up to fp reassociation):
    aggF = segsum(norm * NF[src], dst)                 # matmul commutes with segsum
    x1T  = leaky_relu(W1.T @ aggF.T + b1)              # [HID, N]
    PT   = W2 @ imgT                                   # [HID, B]
    QT   = x1T.T @ PT                                  # [N, B]
    out[:, n] = segsum(norm * QT[src], dst).T + img @ b2

Sharding: nodes (and their incoming edges) are range-sharded across the 8
cores; every segment-sum output is fully core-local.  Three SPMD launches:
  L1: per-core degree -> dinv, output replicated dinv table rows
  L2: layer-1 aggregation (gather + selection matmuls), x1T, PT, QT shard
  L3: final aggregation over QT -> output shard [64, N/8]
Host work between launches is pure layout (concat of shards); all arithmetic
runs on the NeuronCores.  Segment sums are computed as PE matmuls against
one-hot selection matrices built on-device with iota + is_equal, so duplicate
destinations accumulate exactly in fp32 PSUM.
"""

from contextlib import nullcontext

import numpy as np

from concourse import bacc, bass, mybir
from concourse.bass_utils import run_bass_kernel_spmd
from concourse.masks import make_identity
from concourse.tile import TileContext


def _maybe_reps(tc, reps):
    """Hardware repeat loop for timing (reps>1); no-op for production."""
    return tc.For_i(0, reps) if reps > 1 else nullcontext()

P = 128
NB = 125            # nodes per block (psum free dim)
FPAD = 320          # node-feature row padded to 320 f32 = 1280B (256B multiple)
F_TEXT = 300
NEG = 0.2

# full-size problem config
CFG_FULL = dict(B=64, N=20000, E=160000, HID=1024, OUT=1664, CORES=8)

TRACE = False                  # set by test.py for profiling
LAST_EXEC_NS = {}              # launch name -> exec ns (when TRACE)
LAST_BUILD = None              # (nc1, nc2, nc3) from the last kernel() call
LAST_MAPS = None               # {"l1": maps1, "l2": maps2, "l3": maps3}
LAST_REP_BUILDERS = None       # launch name -> (lambda reps: nc), for timing
LAUNCH_ORDER = ["l1", "l2", "l3"]

_BUILD_CACHE = {}


# ----------------------------------------------------------------- host prep

def _prep_edges(edge_src, edge_dst, edge_weight, cfg):
    """Group edges by (core, block) of their dst; pad each block's edge list
    to T_b*128 where T_b is the max tile count for block index b across
    cores (SPMD needs identical program structure on every core)."""
    ncores = cfg["CORES"]
    npc = cfg["N"] // ncores               # nodes per core
    nblk = npc // NB                       # blocks per core
    assert npc % NB == 0

    core = edge_dst // npc
    blk = (edge_dst - core * npc) // NB
    dstl = (edge_dst - core * npc) - blk * NB

    # bucket edge ids per (core, block)
    buckets = [[None] * nblk for _ in range(ncores)]
    order = np.lexsort((blk, core))
    core_s, blk_s = core[order], blk[order]
    bounds = np.searchsorted(core_s * nblk + blk_s, np.arange(ncores * nblk + 1))
    for k in range(ncores):
        for b in range(nblk):
            i0, i1 = bounds[k * nblk + b], bounds[k * nblk + b + 1]
            buckets[k][b] = order[i0:i1]

    TBs = []
    for b in range(nblk):
        mx = max(len(buckets[k][b]) for k in range(ncores))
        TBs.append(max(1, -(-mx // P)))

    per_core = []
    for k in range(ncores):
        srcs, dls, ws = [], [], []
        for b in range(nblk):
            ids = buckets[k][b]
            pad = TBs[b] * P - len(ids)
            srcs.append(np.pad(edge_src[ids], (0, pad)))
            dls.append(np.pad(dstl[ids], (0, pad)))
            ws.append(np.pad(edge_weight[ids], (0, pad)))
        src = np.concatenate(srcs).astype(np.int64)
        dl = np.concatenate(dls).astype(np.float32)
        w = np.concatenate(ws).astype(np.float32)
        dst_glob = np.concatenate(
            [np.pad(edge_dst[buckets[k][b]], (0, TBs[b] * P - len(buckets[k][b])))
             for b in range(nblk)]).astype(np.int64)

        def idx16(a):
            # dma_gather layout: idx j at [j%16, j//16], replicated on 8
            # 16-partition groups -> [128, n/16]
            a16 = a.astype(np.int16).reshape(-1, 16).T
            return np.tile(a16, (8, 1)).copy()

        per_core.append(dict(
            src16=idx16(src),
            dst16=idx16(dst_glob),
            dstl=dl.reshape(-1, P).T.copy(),     # [128, TT]
            wts=w.reshape(-1, P).T.copy(),       # [128, TT]
        ))
    return TBs, per_core


# ------------------------------------------------------------------ builders

def _new_nc():
    return bacc.Bacc(None, target_bir_lowering=False)


def _iota_row(nc, pool):
    """[128, NB] f32 tile with value j in column j (every partition)."""
    ji = pool.tile([P, NB], mybir.dt.int32)
    nc.gpsimd.iota(ji[:], pattern=[[1, NB]], base=0, channel_multiplier=0)
    j = pool.tile([P, NB], mybir.dt.float32)
    nc.vector.tensor_copy(j[:], ji[:])
    return j


def _build_l1(TBs, cfg, reps=1):
    """deg -> dinv -> replicated dinv table rows [npc, 64]."""
    nc = _new_nc()
    nblk = len(TBs)
    TT = sum(TBs)
    npc = cfg["N"] // cfg["CORES"]

    dstl_in = nc.dram_tensor("dstl", [P, TT], mybir.dt.float32, kind="ExternalInput")
    wts_in = nc.dram_tensor("wts", [P, TT], mybir.dt.float32, kind="ExternalInput")
    drep_out = nc.dram_tensor("drep", [npc, 64], mybir.dt.float32, kind="ExternalOutput")

    with TileContext(nc) as tc:
        with (
            tc.tile_pool(name="sbA", bufs=1) as sbA,
            tc.tile_pool(name="sbS", bufs=4) as sbS,
            tc.tile_pool(name="sbR", bufs=2) as sbR,
            tc.tile_pool(name="ps", bufs=2, space="PSUM") as ps,
        ):
            with _maybe_reps(tc, reps):
                J = _iota_row(nc, sbA)
                dstl = sbA.tile([P, TT], mybir.dt.float32)
                wts = sbA.tile([P, TT], mybir.dt.float32)
                nc.sync.dma_start(out=dstl[:], in_=dstl_in[:])
                nc.sync.dma_start(out=wts[:], in_=wts_in[:])

                toff = 0
                for b in range(nblk):
                    Tb = TBs[b]
                    psd = ps.tile([NB, 1], mybir.dt.float32, space="PSUM", tag="deg")
                    for t in range(Tb):
                        col = toff + t
                        S0 = sbS.tile([P, NB], mybir.dt.float32, tag="S0")
                        nc.vector.tensor_scalar(
                            out=S0[:], in0=J[:],
                            scalar1=dstl[:, col:col + 1], scalar2=None,
                            op0=mybir.AluOpType.is_equal)
                        nc.tensor.matmul(out=psd[:], lhsT=S0[:], rhs=wts[:, col:col + 1],
                                         start=(t == 0), stop=(t == Tb - 1))
                    # dinv = sqrt(1/max(deg, deg==0 ? 1)) * (deg > 0)
                    m = sbS.tile([NB, 1], mybir.dt.float32, tag="m")
                    nc.vector.tensor_scalar(out=m[:], in0=psd[:], scalar1=0.0,
                                            scalar2=None, op0=mybir.AluOpType.is_gt)
                    safe = sbS.tile([NB, 1], mybir.dt.float32, tag="safe")
                    le = sbS.tile([NB, 1], mybir.dt.float32, tag="le")
                    nc.vector.tensor_scalar(out=le[:], in0=psd[:], scalar1=0.0,
                                            scalar2=None, op0=mybir.AluOpType.is_le)
                    nc.vector.tensor_tensor(out=safe[:], in0=psd[:], in1=le[:],
                                            op=mybir.AluOpType.add)
                    rec = sbS.tile([NB, 1], mybir.dt.float32, tag="rec")
                    nc.vector.reciprocal(rec[:], safe[:])
                    sq = sbS.tile([NB, 1], mybir.dt.float32, tag="sq")
                    nc.scalar.sqrt(sq[:], rec[:])
                    dv = sbS.tile([NB, 1], mybir.dt.float32, tag="dv")
                    nc.vector.tensor_tensor(out=dv[:], in0=sq[:], in1=m[:],
                                            op=mybir.AluOpType.mult)
                    rep = sbR.tile([NB, 64], mybir.dt.float32, tag="rep")
                    nc.vector.tensor_scalar(out=rep[:], in0=dv[:].to_broadcast([NB, 64]),
                                            scalar1=1.0, scalar2=None,
                                            op0=mybir.AluOpType.mult)
                    nc.sync.dma_start(out=drep_out[b * NB:(b + 1) * NB, :], in_=rep[:])
                    toff += Tb
    nc.finalize()
    return nc


def _build_l2(TBs, cfg, reps=1):
    nc = _new_nc()
    nblk = len(TBs)
    TT = sum(TBs)
    N, HID, OUT, B = cfg["N"], cfg["HID"], cfg["OUT"], cfg["B"]
    npc = N // cfg["CORES"]
    HCH = HID // P          # h chunks
    OCH = OUT // P          # o chunks
    NCW = 500 if npc % 500 == 0 else 250   # x1T n-chunk width
    NCH = npc // NCW

    nf_in = nc.dram_tensor("nf", [N, FPAD], mybir.dt.float32, kind="ExternalInput")
    drep_in = nc.dram_tensor("drep", [N, 64], mybir.dt.float32, kind="ExternalInput")
    w1_in = nc.dram_tensor("w1", [FPAD, HID], mybir.dt.float32r, kind="ExternalInput")
    w2_in = nc.dram_tensor("w2", [HID, OUT], mybir.dt.float32, kind="ExternalInput")
    b1_in = nc.dram_tensor("b1r", [1, HID], mybir.dt.float32r, kind="ExternalInput")
    img_in = nc.dram_tensor("img", [B, OUT], mybir.dt.float32, kind="ExternalInput")
    b2_in = nc.dram_tensor("b2r", [B, OUT], mybir.dt.float32, kind="ExternalInput")
    src_in = nc.dram_tensor("src16", [P, TT * 8], mybir.dt.int16, kind="ExternalInput")
    dsti_in = nc.dram_tensor("dst16", [P, TT * 8], mybir.dt.int16, kind="ExternalInput")
    dstl_in = nc.dram_tensor("dstl", [P, TT], mybir.dt.float32, kind="ExternalInput")
    wts_in = nc.dram_tensor("wts", [P, TT], mybir.dt.float32, kind="ExternalInput")

    qt_out = nc.dram_tensor("qt", [npc, 64], mybir.dt.float32, kind="ExternalOutput")
    c_out = nc.dram_tensor("cvec", [B, 1], mybir.dt.float32, kind="ExternalOutput")
    norm_out = nc.dram_tensor("normv", [P, TT], mybir.dt.float32, kind="ExternalOutput")

    with TileContext(nc) as tc:
        with (
            tc.tile_pool(name="sbA", bufs=1) as sbA,
            tc.tile_pool(name="sbG", bufs=3) as sbG,
            tc.tile_pool(name="sbD", bufs=2) as sbD,
            tc.tile_pool(name="sbS", bufs=4) as sbS,
            tc.tile_pool(name="sbW", bufs=2) as sbW,
            tc.tile_pool(name="ps1", bufs=1, space="PSUM") as ps1,
            tc.tile_pool(name="ps2", bufs=2, space="PSUM") as ps2,
        ):
            with _maybe_reps(tc, reps):
                J = _iota_row(nc, sbA)
                ident = sbA.tile([P, P], mybir.dt.float32)
                make_identity(nc, ident[:])

                # ---------- phase A: W2T / imgT / PT / cvec (overlaps gathers) --
                PTacc = sbA.tile([P, HCH * B], mybir.dt.float32)
                c_sb = sbA.tile([B, 1], mybir.dt.float32)
                with tc.tile_pool(name="sbE", bufs=1) as sbE:
                    imgs = sbE.tile([B, OUT], mybir.dt.float32)
                    nc.sync.dma_start(out=imgs[:], in_=img_in[:])
                    imgT = sbE.tile([P, OCH * B], mybir.dt.float32r)
                    for o in range(OCH):
                        tps = ps2.tile([P, B], mybir.dt.float32, space="PSUM", tag="tr")
                        nc.tensor.transpose(tps[:], imgs[:, o * P:(o + 1) * P],
                                            ident[:B, :B])
                        nc.vector.tensor_copy(imgT[:, o * B:(o + 1) * B], tps[:])
                    b2r = sbE.tile([B, OUT], mybir.dt.float32)
                    nc.sync.dma_start(out=b2r[:], in_=b2_in[:])
                    nc.vector.tensor_tensor(out=b2r[:], in0=imgs[:], in1=b2r[:],
                                            op=mybir.AluOpType.mult)
                    nc.vector.tensor_reduce(out=c_sb[:], in_=b2r[:],
                                            axis=mybir.AxisListType.X,
                                            op=mybir.AluOpType.add)
                    nc.sync.dma_start(out=c_out[:], in_=c_sb[:])

                    for o in range(OCH):
                        w2src = sbE.tile([P, HCH, P], mybir.dt.float32, tag="w2src",
                                         bufs=1)
                        nc.sync.dma_start(
                            out=w2src[:],
                            in_=bass.AP(w2_in, o * P,
                                        [[OUT, P], [OUT * P, HCH], [1, P]]))
                        w2T = sbE.tile([P, HID], mybir.dt.float32r, tag="w2T", bufs=1)
                        for h in range(HCH):
                            tps = ps2.tile([P, P], mybir.dt.float32, space="PSUM",
                                           tag="tr")
                            nc.tensor.transpose(tps[:], w2src[:, h, :], ident[:])
                            nc.vector.tensor_copy(w2T[:, h * P:(h + 1) * P], tps[:])
                        iT = imgT[:, o * B:(o + 1) * B]
                        pt_ps = ps1.tile([P, HCH * B], mybir.dt.float32, space="PSUM",
                                         tag="pt", name="pt_ps")
                        for h in range(HCH):
                            nc.tensor.matmul(out=pt_ps[:, h * B:(h + 1) * B],
                                             lhsT=w2T[:, h * P:(h + 1) * P], rhs=iT,
                                             start=True, stop=True)
                        if o == 0:
                            nc.vector.tensor_copy(PTacc[:], pt_ps[:])
                        else:
                            nc.vector.tensor_tensor(out=PTacc[:], in0=PTacc[:],
                                                    in1=pt_ps[:],
                                                    op=mybir.AluOpType.add)
                PT = sbA.tile([P, HCH * B], mybir.dt.bfloat16)
                nc.vector.tensor_copy(PT[:], PTacc[:])

                # ---------- phase B: gathers + layer-1 aggregation -------------
                src16 = sbA.tile([P, TT * 8], mybir.dt.int16)
                dst16 = sbA.tile([P, TT * 8], mybir.dt.int16)
                dstl = sbA.tile([P, TT], mybir.dt.float32)
                wts = sbA.tile([P, TT], mybir.dt.float32)
                nc.sync.dma_start(out=src16[:], in_=src_in[:])
                nc.sync.dma_start(out=dst16[:], in_=dsti_in[:])
                nc.sync.dma_start(out=dstl[:], in_=dstl_in[:])
                nc.sync.dma_start(out=wts[:], in_=wts_in[:])

                agg = [sbA.tile([P, npc], mybir.dt.float32r, tag=f"agg{i}",
                                name=f"agg{i}") for i in range(2)]
                agg.append(sbA.tile([64, npc], mybir.dt.float32r, tag="agg2",
                                    name="agg2"))
                norm_all = sbA.tile([P, TT], mybir.dt.float32)

                toff = 0
                for b in range(nblk):
                    Tb = TBs[b]
                    ni = Tb * P
                    # dinv gathers for this block's src and dst
                    dsg = sbD.tile([P, Tb, 64], mybir.dt.float32, tag="dsg")
                    ddg = sbD.tile([P, Tb, 64], mybir.dt.float32, tag="ddg")
                    nc.gpsimd.dma_gather(
                        out_ap=dsg[:], in_ap=drep_in[:],
                        idxs_ap=src16[:, toff * 8:(toff + Tb) * 8],
                        num_idxs=ni, num_idxs_reg=ni, elem_size=64, single_packet=False)
                    nc.gpsimd.dma_gather(
                        out_ap=ddg[:], in_ap=drep_in[:],
                        idxs_ap=dst16[:, toff * 8:(toff + Tb) * 8],
                        num_idxs=ni, num_idxs_reg=ni, elem_size=64, single_packet=False)
                    # norm = dinv_src * w * dinv_dst  [128, Tb]
                    nrm = norm_all[:, toff:toff + Tb]
                    nc.vector.tensor_tensor(
                        out=nrm, in0=dsg[:, :, 0],
                        in1=wts[:, toff:toff + Tb], op=mybir.AluOpType.mult)
                    nc.vector.tensor_tensor(
                        out=nrm, in0=nrm, in1=ddg[:, :, 0], op=mybir.AluOpType.mult)

                    # message gather + bf16 cast
                    mg = sbG.tile([P, Tb, FPAD], mybir.dt.float32, tag="mg")
                    nc.gpsimd.dma_gather(
                        out_ap=mg[:], in_ap=nf_in[:],
                        idxs_ap=src16[:, toff * 8:(toff + Tb) * 8],
                        num_idxs=ni, num_idxs_reg=ni, elem_size=FPAD, single_packet=False)
                    m16 = sbS.tile([P, Tb * FPAD], mybir.dt.bfloat16, tag="m16", bufs=2)
                    nc.scalar.activation(
                        out=m16[:], in_=mg[:].rearrange("p c d -> p (c d)"),
                        func=mybir.ActivationFunctionType.Copy)

                    pa = [ps1.tile([P, NB], mybir.dt.float32, space="PSUM", tag="pa0", name="pa0"),
                          ps1.tile([P, NB], mybir.dt.float32, space="PSUM", tag="pa1", name="pa1"),
                          ps1.tile([64, NB], mybir.dt.float32, space="PSUM", tag="pa2", name="pa2")]
                    for t in range(Tb):
                        col = toff + t
                        S0 = sbS.tile([P, NB], mybir.dt.bfloat16, tag="S0")
                        nc.vector.tensor_scalar(
                            out=S0[:], in0=J[:], scalar1=dstl[:, col:col + 1],
                            scalar2=None, op0=mybir.AluOpType.is_equal)
                        S1 = sbS.tile([P, NB], mybir.dt.bfloat16, tag="S1")
                        nc.vector.tensor_scalar(
                            out=S1[:], in0=S0[:], scalar1=norm_all[:, col:col + 1],
                            scalar2=None, op0=mybir.AluOpType.mult)
                        for fc, fw in ((0, P), (1, P), (2, 64)):
                            nc.tensor.matmul(
                                out=pa[fc][:, :],
                                lhsT=m16[:, t * FPAD + fc * P: t * FPAD + fc * P + fw],
                                rhs=S1[:],
                                start=(t == 0), stop=(t == Tb - 1))
                    for fc in range(3):
                        nc.vector.tensor_copy(agg[fc][:, b * NB:(b + 1) * NB], pa[fc][:])
                    toff += Tb
                nc.sync.dma_start(out=norm_out[:], in_=norm_all[:])

                # ---------- phase C: x1T = leaky(W1.T @ aggF + b1) --------------
                w1t = [sbA.tile([P, HID], mybir.dt.float32r, tag="w1k0", name="w1k0"),
                       sbA.tile([P, HID], mybir.dt.float32r, tag="w1k1", name="w1k1"),
                       sbA.tile([64, HID], mybir.dt.float32r, tag="w1k2", name="w1k2")]
                nc.sync.dma_start(out=w1t[0][:], in_=w1_in[0:P, :])
                nc.sync.dma_start(out=w1t[1][:], in_=w1_in[P:2 * P, :])
                nc.sync.dma_start(out=w1t[2][:], in_=w1_in[2 * P:2 * P + 64, :])
                b1row = sbA.tile([1, HID], mybir.dt.float32r)
                nc.sync.dma_start(out=b1row[:], in_=b1_in[:])
                ones_f = sbA.tile([1, npc], mybir.dt.float32)
                nc.vector.memset(ones_f[:], 1.0)
                ones = sbA.tile([1, npc], mybir.dt.float32r)
                nc.vector.tensor_copy(ones[:], ones_f[:])
                x1T = [sbA.tile([P, npc], mybir.dt.bfloat16, tag=f"x1T{h}",
                                name=f"x1T{h}") for h in range(HCH)]
                for h in range(HCH):
                    for nchi in range(NCH):
                        n0 = nchi * NCW
                        px = ps2.tile([P, NCW], mybir.dt.float32, space="PSUM", tag="xq")
                        for kc in range(3):
                            nc.tensor.matmul(
                                out=px[:],
                                lhsT=w1t[kc][:, h * P:(h + 1) * P],
                                rhs=agg[kc][:, n0:n0 + NCW],
                                start=(kc == 0), stop=False)
                        nc.tensor.matmul(
                            out=px[:], lhsT=b1row[:, h * P:(h + 1) * P],
                            rhs=ones[:, n0:n0 + NCW], start=False, stop=True)
                        t2 = sbS.tile([P, NCW], mybir.dt.float32, tag="t2", bufs=2)
                        nc.scalar.activation(out=t2[:], in_=px[:],
                                             func=mybir.ActivationFunctionType.Copy,
                                             scale=NEG)
                        nc.vector.tensor_tensor(out=x1T[h][:, n0:n0 + NCW],
                                                in0=px[:], in1=t2[:],
                                                op=mybir.AluOpType.max)

                # ---------- phase D: QT = x1T.T @ PT ---------------------------
                for nchi in range(npc // NB):
                    n0 = nchi * NB
                    pq = ps2.tile([NB, B], mybir.dt.float32, space="PSUM", tag="xq")
                    for h in range(HCH):
                        nc.tensor.matmul(
                            out=pq[:], lhsT=x1T[h][:, n0:n0 + NB],
                            rhs=PT[:, h * B:(h + 1) * B],
                            start=(h == 0), stop=(h == HCH - 1))
                    qsb = sbS.tile([NB, B], mybir.dt.float32, tag="qsb")
                    nc.vector.tensor_copy(qsb[:], pq[:])
                    nc.sync.dma_start(out=qt_out[n0:n0 + NB, :], in_=qsb[:])
    nc.finalize()
    return nc


def _build_l3(TBs, cfg, reps=1):
    nc = _new_nc()
    nblk = len(TBs)
    TT = sum(TBs)
    N, B = cfg["N"], cfg["B"]
    npc = N // cfg["CORES"]

    qt_in = nc.dram_tensor("qt", [N, 64], mybir.dt.float32, kind="ExternalInput")
    src_in = nc.dram_tensor("src16", [P, TT * 8], mybir.dt.int16, kind="ExternalInput")
    dstl_in = nc.dram_tensor("dstl", [P, TT], mybir.dt.float32, kind="ExternalInput")
    nrm_in = nc.dram_tensor("normv", [P, TT], mybir.dt.float32, kind="ExternalInput")
    c_in = nc.dram_tensor("cvec", [B, 1], mybir.dt.float32, kind="ExternalInput")
    out_own = nc.dram_tensor("outp", [B, npc], mybir.dt.float32, kind="ExternalOutput")

    with TileContext(nc) as tc:
        with (
            tc.tile_pool(name="sbA", bufs=1) as sbA,
            tc.tile_pool(name="sbG", bufs=3) as sbG,
            tc.tile_pool(name="sbS", bufs=4) as sbS,
            tc.tile_pool(name="ps", bufs=2, space="PSUM") as ps,
        ):
            with _maybe_reps(tc, reps):
                J = _iota_row(nc, sbA)
                src16 = sbA.tile([P, TT * 8], mybir.dt.int16)
                dstl = sbA.tile([P, TT], mybir.dt.float32)
                nrm = sbA.tile([P, TT], mybir.dt.float32)
                c_sb = sbA.tile([B, 1], mybir.dt.float32)
                nc.sync.dma_start(out=src16[:], in_=src_in[:])
                nc.sync.dma_start(out=dstl[:], in_=dstl_in[:])
                nc.sync.dma_start(out=nrm[:], in_=nrm_in[:])
                nc.sync.dma_start(out=c_sb[:], in_=c_in[:])

                toff = 0
                for b in range(nblk):
                    Tb = TBs[b]
                    ni = Tb * P
                    qg = sbG.tile([P, Tb, 64], mybir.dt.float32, tag="qg")
                    nc.gpsimd.dma_gather(
                        out_ap=qg[:], in_ap=qt_in[:],
                        idxs_ap=src16[:, toff * 8:(toff + Tb) * 8],
                        num_idxs=ni, num_idxs_reg=ni, elem_size=64, single_packet=False)
                    q16 = sbS.tile([P, Tb * 64], mybir.dt.bfloat16, tag="q16")
                    nc.scalar.activation(
                        out=q16[:], in_=qg[:].rearrange("p c d -> p (c d)"),
                        func=mybir.ActivationFunctionType.Copy)
                    po = ps.tile([B, NB], mybir.dt.float32, space="PSUM", tag="po")
                    for t in range(Tb):
                        col = toff + t
                        S0 = sbS.tile([P, NB], mybir.dt.bfloat16, tag="S0")
                        nc.vector.tensor_scalar(
                            out=S0[:], in0=J[:], scalar1=dstl[:, col:col + 1],
                            scalar2=None, op0=mybir.AluOpType.is_equal)
                        S1 = sbS.tile([P, NB], mybir.dt.bfloat16, tag="S1")
                        nc.vector.tensor_scalar(
                            out=S1[:], in0=S0[:], scalar1=nrm[:, col:col + 1],
                            scalar2=None, op0=mybir.AluOpType.mult)
                        nc.tensor.matmul(out=po[:],
                                         lhsT=q16[:, t * 64:t * 64 + B],
                                         rhs=S1[:],
                                         start=(t == 0), stop=(t == Tb - 1))
                    osb = sbS.tile([B, NB], mybir.dt.float32, tag="osb")
                    nc.vector.tensor_scalar(out=osb[:], in0=po[:], scalar1=c_sb[:, 0:1],
                                            scalar2=None, op0=mybir.AluOpType.add)
                    nc.sync.dma_start(out=out_own[:, b * NB:(b + 1) * NB], in_=osb[:])
                    toff += Tb
    nc.finalize()
    return nc


# ------------------------------------------------------------------- runner

def _run(name, nc, in_maps, cores):
    kw = {}
    if TRACE:
        kw = dict(trace=True)
    res = run_bass_kernel_spmd(nc, in_maps, core_ids=list(range(cores)), **kw)
    if res.exec_time_ns is not None:
        LAST_EXEC_NS[name] = res.exec_time_ns
    return res.results


def _kernel_impl(img_feat, node_features, edge_src, edge_dst, edge_weight,
                 W1, b1, W2, b2, cfg):
    ncores = cfg["CORES"]
    N, B, HID, OUT = cfg["N"], cfg["B"], cfg["HID"], cfg["OUT"]
    npc = N // ncores
    OCH = OUT // P

    TBs, per_core = _prep_edges(edge_src, edge_dst, edge_weight, cfg)
    key = (tuple(TBs), tuple(sorted(cfg.items())))
    if key not in _BUILD_CACHE:
        _BUILD_CACHE[key] = (_build_l1(TBs, cfg), _build_l2(TBs, cfg),
                             _build_l3(TBs, cfg))
    nc1, nc2, nc3 = _BUILD_CACHE[key]

    # ---- L1: dinv replicated table
    maps1 = [{"dstl": pc["dstl"], "wts": pc["wts"]} for pc in per_core]
    r1 = _run("l1", nc1, maps1, ncores)
    drep = np.concatenate([r1[k]["drep"] for k in range(ncores)], axis=0)

    # ---- L2
    nf_pad = np.zeros((N, FPAD), np.float32)
    nf_pad[:, :F_TEXT] = node_features
    w1_pad = np.zeros((FPAD, HID), np.float32)
    w1_pad[:F_TEXT, :] = W1
    b2rep = np.ascontiguousarray(np.broadcast_to(b2, (B, OUT))).astype(np.float32)
    maps2 = [dict(nf=nf_pad, drep=drep, w1=w1_pad, w2=W2, img=img_feat,
                  b1r=b1.reshape(1, -1).astype(np.float32), b2r=b2rep,
                  src16=pc["src16"], dst16=pc["dst16"],
                  dstl=pc["dstl"], wts=pc["wts"]) for pc in per_core]
    r2 = _run("l2", nc2, maps2, ncores)
    qt = np.concatenate([r2[k]["qt"] for k in range(ncores)], axis=0)
    cvec = r2[0]["cvec"]
    norms = [r2[k]["normv"] for k in range(ncores)]

    # ---- L3
    maps3 = [dict(qt=qt, src16=pc["src16"], dstl=pc["dstl"],
                  normv=norms[k], cvec=cvec)
             for k, pc in enumerate(per_core)]
    r3 = _run("l3", nc3, maps3, ncores)
    out = np.concatenate([r3[k]["outp"] for k in range(ncores)], axis=1)

    global LAST_BUILD, LAST_MAPS, LAST_REP_BUILDERS
    LAST_BUILD = (nc1, nc2, nc3)
    LAST_MAPS = {"l1": maps1, "l2": maps2, "l3": maps3}
    LAST_REP_BUILDERS = {
        "l1": lambda reps: _build_l1(TBs, cfg, reps=reps),
        "l2": lambda reps: _build_l2(TBs, cfg, reps=reps),
        "l3": lambda reps: _build_l3(TBs, cfg, reps=reps),
    }
    return out.astype(np.float32)


def kernel(img_feat, node_features, edge_src, edge_dst, edge_weight,
           W1, b1, W2, b2):
    return _kernel_impl(np.asarray(img_feat), np.asarray(node_features),
                        np.asarray(edge_src), np.asarray(edge_dst),
                        np.asarray(edge_weight), np.asarray(W1),
                        np.asarray(b1), np.asarray(W2), np.asarray(b2),
                        CFG_FULL)



# revision 9
# speedup vs baseline: 3.5266x; 1.5088x over previous
"""Trainium2 Bass kernel for nn_BaseGNNModel (2-layer GCN + image-query matmul).

Math (reference):
    norm = dinv[src] * w * dinv[dst],  dinv = rsqrt(segment_sum(w, dst))
    x1 = leaky_relu(segsum(norm * (NF @ W1)[src], dst) + b1, 0.2)
    x2 = segsum(norm * (x1 @ W2)[src], dst) + b2
    out = img @ x2.T                                  # [64, 20000]

Algebraic restructure (exact up to fp reassociation), with D = diag(dinv),
A_w the weighted adjacency and nhat = D A_w D:
    aggF = D A_w (D NF)        -- src-side D pre-folded into the NF table,
                                  dst-side D applied per dst-node column
    x1T  = lrelu(W1.T @ aggF.T + b1)                  # [HID, N]
    PT   = W2 @ imgT                                  # [HID, B]
    Qs   = D (x1T.T @ PT)                             # [N, B] pre-scaled
    out  = (A_w Qs scaled by D on dst).T + img @ b2

Sharding: nodes (and their incoming edges) range-sharded across 8 cores;
segment sums are fully core-local (PE matmuls against one-hot selection
matrices S1 = (iota == dst_local) * w).  Three SPMD launches:
  L1: deg -> dinv; S1 matrices (stored to HBM for reuse by L2/L3); the
      bf16 dinv-scaled NF gather table; per-core partial PT over a 208-col
      slice of W2; partial img@b2.
  L2: gather NF table rows by edge src, aggregate via S1, dst-scale,
      x1T = lrelu(...), Qs = dinv * (x1 @ PT)  (bf16 gather table out)
  L3: gather Qs rows by src, aggregate via S1, dst-scale, + img@b2
Host work between launches is pure layout (concat / transpose of shards).
"""

from contextlib import nullcontext

import numpy as np

from concourse import bacc, bass, mybir
from concourse.bass_utils import run_bass_kernel_spmd
from concourse.masks import make_identity
from concourse.tile import TileContext


def _maybe_reps(tc, reps):
    """Hardware repeat loop for timing (reps>1); no-op for production."""
    return tc.For_i(0, reps) if reps > 1 else nullcontext()

P = 128
NB = 125            # nodes per block (psum free dim)
F_TEXT = 300
FPAD = 320          # W1 rows padded (300 -> 320 = 128+128+64 chunks)
NFS_W = 384         # bf16 NF-table row: 384*2B = 768B (256B multiple)
QS_W = 128          # bf16 Qs-table row: 128*2B = 256B
NEG = 0.2
FCH = ((0, P), (P, P), (2 * P, 64))   # feature chunks of FPAD

# full-size problem config
CFG_FULL = dict(B=64, N=20000, E=160000, HID=1024, OUT=1664, CORES=8)

TRACE = False                  # set by test.py for profiling
LAST_EXEC_NS = {}              # launch name -> exec ns (when TRACE)
LAST_BUILD = None              # (nc1, nc2, nc3) from the last kernel() call
LAST_MAPS = None               # {"l1": maps1, "l2": maps2, "l3": maps3}
LAST_REP_BUILDERS = None       # launch name -> (lambda reps: nc), for timing
LAUNCH_ORDER = ["l1", "l2", "l3"]

_BUILD_CACHE = {}


# ----------------------------------------------------------------- host prep

def _prep_edges(edge_src, edge_dst, edge_weight, cfg):
    """Group edges by (core, block) of their dst; pad each block's edge list
    to T_b*128 where T_b is the max tile count for block index b across
    cores (SPMD needs identical program structure on every core)."""
    ncores = cfg["CORES"]
    npc = cfg["N"] // ncores               # nodes per core
    nblk = npc // NB                       # blocks per core
    assert npc % NB == 0

    core = edge_dst // npc
    blk = (edge_dst - core * npc) // NB
    dstl = (edge_dst - core * npc) - blk * NB

    # bucket edge ids per (core, block)
    buckets = [[None] * nblk for _ in range(ncores)]
    order = np.lexsort((blk, core))
    core_s, blk_s = core[order], blk[order]
    bounds = np.searchsorted(core_s * nblk + blk_s, np.arange(ncores * nblk + 1))
    for k in range(ncores):
        for b in range(nblk):
            i0, i1 = bounds[k * nblk + b], bounds[k * nblk + b + 1]
            buckets[k][b] = order[i0:i1]

    TBs = []
    for b in range(nblk):
        mx = max(len(buckets[k][b]) for k in range(ncores))
        TBs.append(max(1, -(-mx // P)))

    per_core = []
    for k in range(ncores):
        srcs, dls, ws = [], [], []
        for b in range(nblk):
            ids = buckets[k][b]
            pad = TBs[b] * P - len(ids)
            srcs.append(np.pad(edge_src[ids], (0, pad)))
            dls.append(np.pad(dstl[ids], (0, pad)))
            ws.append(np.pad(edge_weight[ids], (0, pad)))
        src = np.concatenate(srcs).astype(np.int64)
        dl = np.concatenate(dls).astype(np.float32)
        w = np.concatenate(ws).astype(np.float32)

        def idx16(a):
            # dma_gather layout: idx j at [j%16, j//16], replicated on 8
            # 16-partition groups -> [128, n/16]
            a16 = a.astype(np.int16).reshape(-1, 16).T
            return np.tile(a16, (8, 1)).copy()

        per_core.append(dict(
            src16=idx16(src),
            dstl=dl.reshape(-1, P).T.copy(),     # [128, TT]
            wts=w.reshape(-1, P).T.copy(),       # [128, TT]
        ))
    return TBs, per_core


# ------------------------------------------------------------------ builders

def _new_nc():
    return bacc.Bacc(None, target_bir_lowering=False)


def _iota_row(nc, pool):
    """[128, NB] f32 tile with value j in column j (every partition)."""
    ji = pool.tile([P, NB], mybir.dt.int32)
    nc.gpsimd.iota(ji[:], pattern=[[1, NB]], base=0, channel_multiplier=0)
    j = pool.tile([P, NB], mybir.dt.float32)
    nc.vector.tensor_copy(j[:], ji[:])
    return j


def _build_l1(TBs, cfg, reps=1):
    """deg -> dinv; S1 selection matrices; bf16 dinv*NF table; partial PT."""
    nc = _new_nc()
    nblk = len(TBs)
    TT = sum(TBs)
    N, B, HID, OUT = cfg["N"], cfg["B"], cfg["HID"], cfg["OUT"]
    npc = N // cfg["CORES"]
    OSL = OUT // cfg["CORES"]              # 208 = 128 + 80
    HCH = HID // P

    dstl_in = nc.dram_tensor("dstl", [P, TT], mybir.dt.float32, kind="ExternalInput")
    wts_in = nc.dram_tensor("wts", [P, TT], mybir.dt.float32, kind="ExternalInput")
    nfsl_in = nc.dram_tensor("nfslab", [npc, F_TEXT], mybir.dt.float32,
                             kind="ExternalInput")
    w2ts_in = nc.dram_tensor("w2ts", [OSL, HID], mybir.dt.float32,
                             kind="ExternalInput")
    imgts_in = nc.dram_tensor("imgts", [OSL, B], mybir.dt.float32,
                              kind="ExternalInput")
    b2s_in = nc.dram_tensor("b2s", [OSL, 1], mybir.dt.float32,
                            kind="ExternalInput")

    nfs_out = nc.dram_tensor("nfs", [npc, NFS_W], mybir.dt.bfloat16,
                             kind="ExternalOutput")
    s1_out = nc.dram_tensor("s1", [P, TT * NB], mybir.dt.bfloat16,
                            kind="ExternalOutput")
    dv2d_out = nc.dram_tensor("dv2d", [NB, nblk], mybir.dt.float32,
                              kind="ExternalOutput")
    dvrow_out = nc.dram_tensor("dvrow", [1, npc], mybir.dt.float32,
                               kind="ExternalOutput")
    ptp_out = nc.dram_tensor("ptp", [HID, B], mybir.dt.float32,
                             kind="ExternalOutput")
    cvp_out = nc.dram_tensor("cvp", [B, 1], mybir.dt.float32,
                             kind="ExternalOutput")

    with TileContext(nc) as tc:
        with (
            tc.tile_pool(name="sbA", bufs=1) as sbA,
            tc.tile_pool(name="sbB", bufs=2) as sbB,
            tc.tile_pool(name="sbN", bufs=3) as sbN,
            tc.tile_pool(name="sbS", bufs=4) as sbS,
            tc.tile_pool(name="ps", bufs=2, space="PSUM") as ps,
        ):
            with _maybe_reps(tc, reps):
                J = _iota_row(nc, sbA)
                identNB = sbA.tile([NB, NB], mybir.dt.float32)
                make_identity(nc, identNB[:])
                ones_bf = sbA.tile([P, 1], mybir.dt.bfloat16)
                nc.vector.memset(ones_bf[:], 1.0)

                dstl = sbA.tile([P, TT], mybir.dt.float32)
                wts = sbA.tile([P, TT], mybir.dt.float32)
                nc.sync.dma_start(out=dstl[:], in_=dstl_in[:])
                nc.sync.dma_start(out=wts[:], in_=wts_in[:])

                # ---- partial PT = W2[:, oslice] @ imgT[oslice] and img@b2 --
                w2a = sbA.tile([P, HID], mybir.dt.float32)
                w2b = sbA.tile([OSL - P, HID], mybir.dt.float32)
                nc.sync.dma_start(out=w2a[:], in_=w2ts_in[0:P, :])
                nc.sync.dma_start(out=w2b[:], in_=w2ts_in[P:OSL, :])
                imga = sbA.tile([P, B], mybir.dt.float32)
                imgb = sbA.tile([OSL - P, B], mybir.dt.float32)
                nc.sync.dma_start(out=imga[:], in_=imgts_in[0:P, :])
                nc.sync.dma_start(out=imgb[:], in_=imgts_in[P:OSL, :])
                b2a = sbA.tile([P, 1], mybir.dt.float32)
                b2b = sbA.tile([OSL - P, 1], mybir.dt.float32)
                nc.sync.dma_start(out=b2a[:], in_=b2s_in[0:P, :])
                nc.sync.dma_start(out=b2b[:], in_=b2s_in[P:OSL, :])

                for h in range(HCH):
                    ptps = ps.tile([P, B], mybir.dt.float32, space="PSUM",
                                   tag="pt")
                    nc.tensor.matmul(out=ptps[:], lhsT=w2a[:, h * P:(h + 1) * P],
                                     rhs=imga[:], start=True, stop=False)
                    nc.tensor.matmul(out=ptps[:], lhsT=w2b[:, h * P:(h + 1) * P],
                                     rhs=imgb[:], start=False, stop=True)
                    ptsb = sbS.tile([P, B], mybir.dt.float32, tag="ptsb")
                    nc.vector.tensor_copy(ptsb[:], ptps[:])
                    nc.sync.dma_start(out=ptp_out[h * P:(h + 1) * P, :],
                                      in_=ptsb[:])
                cvps = ps.tile([B, 1], mybir.dt.float32, space="PSUM", tag="cv")
                nc.tensor.matmul(out=cvps[:], lhsT=imga[:], rhs=b2a[:],
                                 start=True, stop=False)
                nc.tensor.matmul(out=cvps[:], lhsT=imgb[:], rhs=b2b[:],
                                 start=False, stop=True)
                cvsb = sbS.tile([B, 1], mybir.dt.float32, tag="cvsb")
                nc.vector.tensor_copy(cvsb[:], cvps[:])
                nc.sync.dma_start(out=cvp_out[:], in_=cvsb[:])

                # ---- per block: S1, deg -> dinv, scaled NF rows ------------
                dvsb = sbA.tile([NB, nblk], mybir.dt.float32)
                dvrsb = sbA.tile([1, npc], mybir.dt.float32)

                toff = 0
                for b in range(nblk):
                    Tb = TBs[b]
                    s1b = sbB.tile([P, Tb * NB], mybir.dt.bfloat16, tag="s1b")
                    psd = ps.tile([NB, 1], mybir.dt.float32, space="PSUM",
                                  tag="deg")
                    for t in range(Tb):
                        col = toff + t
                        s1t = s1b[:, t * NB:(t + 1) * NB]
                        nc.vector.tensor_scalar(
                            out=s1t, in0=J[:],
                            scalar1=dstl[:, col:col + 1],
                            scalar2=wts[:, col:col + 1],
                            op0=mybir.AluOpType.is_equal,
                            op1=mybir.AluOpType.mult)
                        nc.tensor.matmul(out=psd[:], lhsT=s1t, rhs=ones_bf[:],
                                         start=(t == 0), stop=(t == Tb - 1))
                    nc.sync.dma_start(
                        out=s1_out[:, toff * NB:(toff + Tb) * NB], in_=s1b[:])

                    # dinv = sqrt(1/max(deg, deg==0 ? 1)) * (deg > 0)
                    m = sbS.tile([NB, 1], mybir.dt.float32, tag="m")
                    nc.vector.tensor_scalar(out=m[:], in0=psd[:], scalar1=0.0,
                                            scalar2=None,
                                            op0=mybir.AluOpType.is_gt)
                    le = sbS.tile([NB, 1], mybir.dt.float32, tag="le")
                    nc.vector.tensor_scalar(out=le[:], in0=psd[:], scalar1=0.0,
                                            scalar2=None,
                                            op0=mybir.AluOpType.is_le)
                    safe = sbS.tile([NB, 1], mybir.dt.float32, tag="safe")
                    nc.vector.tensor_tensor(out=safe[:], in0=psd[:], in1=le[:],
                                            op=mybir.AluOpType.add)
                    rec = sbS.tile([NB, 1], mybir.dt.float32, tag="rec")
                    nc.vector.reciprocal(rec[:], safe[:])
                    sq = sbS.tile([NB, 1], mybir.dt.float32, tag="sq")
                    nc.scalar.sqrt(sq[:], rec[:])
                    dv = sbS.tile([NB, 1], mybir.dt.float32, tag="dv")
                    nc.vector.tensor_tensor(out=dv[:], in0=sq[:], in1=m[:],
                                            op=mybir.AluOpType.mult)
                    nc.vector.tensor_copy(dvsb[:, b:b + 1], dv[:])
                    # dv as a row: dvrow[0, b*NB:(b+1)*NB] = dv.T
                    rps = ps.tile([1, NB], mybir.dt.float32, space="PSUM",
                                  tag="dvr")
                    nc.tensor.matmul(out=rps[:], lhsT=dv[:], rhs=identNB[:],
                                     start=True, stop=True)
                    nc.vector.tensor_copy(dvrsb[:, b * NB:(b + 1) * NB], rps[:])

                    # bf16 dinv-scaled NF rows for this block
                    nfb = sbN.tile([NB, F_TEXT], mybir.dt.float32, tag="nfb")
                    nc.sync.dma_start(out=nfb[:],
                                      in_=nfsl_in[b * NB:(b + 1) * NB, :])
                    nfsb = sbN.tile([NB, NFS_W], mybir.dt.bfloat16, tag="nfsb")
                    nc.vector.memset(nfsb[:, F_TEXT:NFS_W], 0.0)
                    nc.vector.tensor_scalar(out=nfsb[:, 0:F_TEXT], in0=nfb[:],
                                            scalar1=dv[:], scalar2=None,
                                            op0=mybir.AluOpType.mult)
                    nc.sync.dma_start(out=nfs_out[b * NB:(b + 1) * NB, :],
                                      in_=nfsb[:])
                    toff += Tb
                nc.sync.dma_start(out=dv2d_out[:], in_=dvsb[:])
                nc.sync.dma_start(out=dvrow_out[:], in_=dvrsb[:])
    nc.finalize()
    return nc


def _build_l2(TBs, cfg, reps=1):
    """Layer-1 aggregation + x1T + Qs table."""
    nc = _new_nc()
    nblk = len(TBs)
    TT = sum(TBs)
    N, B, HID, OUT = cfg["N"], cfg["B"], cfg["HID"], cfg["OUT"]
    ncores = cfg["CORES"]
    npc = N // ncores
    HCH = HID // P          # 8
    NCW = 500               # x1T n-chunk width (f32r full-rate needs >=256)
    NCH = npc // NCW

    nfs_in = nc.dram_tensor("nfs", [N, NFS_W], mybir.dt.bfloat16,
                            kind="ExternalInput")
    s1_in = nc.dram_tensor("s1", [P, TT * NB], mybir.dt.bfloat16,
                           kind="ExternalInput")
    src_in = nc.dram_tensor("src16", [P, TT * 8], mybir.dt.int16,
                            kind="ExternalInput")
    dv2d_in = nc.dram_tensor("dv2d", [NB, nblk], mybir.dt.float32,
                             kind="ExternalInput")
    dvrow_in = nc.dram_tensor("dvrow", [1, npc], mybir.dt.float32,
                              kind="ExternalInput")
    ptp_in = nc.dram_tensor("ptp", [HID, ncores * B], mybir.dt.float32,
                            kind="ExternalInput")
    w1_in = nc.dram_tensor("w1", [FPAD, HID], mybir.dt.float32r,
                           kind="ExternalInput")
    b1c_in = nc.dram_tensor("b1c", [P, HCH], mybir.dt.float32,
                            kind="ExternalInput")

    qs_out = nc.dram_tensor("qs", [npc, QS_W], mybir.dt.bfloat16,
                            kind="ExternalOutput")

    with TileContext(nc) as tc:
        with (
            tc.tile_pool(name="sbA", bufs=1) as sbA,
            tc.tile_pool(name="sbG", bufs=3) as sbG,
            tc.tile_pool(name="sbS", bufs=4) as sbS,
            tc.tile_pool(name="ps1", bufs=1, space="PSUM") as ps1,
            tc.tile_pool(name="ps2", bufs=2, space="PSUM") as ps2,
        ):
            with _maybe_reps(tc, reps):
                s1_all = sbA.tile([P, TT * NB], mybir.dt.bfloat16)
                src16 = sbA.tile([P, TT * 8], mybir.dt.int16)
                dv2d = sbA.tile([NB, nblk], mybir.dt.float32)
                dvrow = sbA.tile([1, npc], mybir.dt.float32)
                b1c = sbA.tile([P, HCH], mybir.dt.float32)
                nc.sync.dma_start(out=s1_all[:], in_=s1_in[:])
                nc.sync.dma_start(out=src16[:], in_=src_in[:])
                nc.sync.dma_start(out=dv2d[:], in_=dv2d_in[:])
                nc.sync.dma_start(out=dvrow[:], in_=dvrow_in[:])
                nc.sync.dma_start(out=b1c[:], in_=b1c_in[:])
                ones_row = sbA.tile([1, P], mybir.dt.float32)
                nc.vector.memset(ones_row[:], 1.0)
                alph = sbA.tile([P, 1], mybir.dt.float32)
                nc.vector.memset(alph[:], NEG)

                # ---- PT = sum of the 8 per-core partials; bf16 ------------
                PTf = sbA.tile([P, HCH * B], mybir.dt.float32)
                with tc.tile_pool(name="sbE", bufs=2) as sbE:
                    for h in range(HCH):
                        pch = sbE.tile([P, ncores * B], mybir.dt.float32,
                                       tag="pch")
                        nc.sync.dma_start(out=pch[:],
                                          in_=ptp_in[h * P:(h + 1) * P, :])
                        acc = PTf[:, h * B:(h + 1) * B]
                        nc.vector.tensor_copy(acc, pch[:, 0:B])
                        for j in range(1, ncores):
                            nc.vector.tensor_tensor(
                                out=acc, in0=acc, in1=pch[:, j * B:(j + 1) * B],
                                op=mybir.AluOpType.add)
                PT = sbA.tile([P, HCH * B], mybir.dt.bfloat16)
                nc.vector.tensor_copy(PT[:], PTf[:])

                # ---- W1 chunks --------------------------------------------
                w1t = [sbA.tile([P, HID], mybir.dt.float32r, name="w1k0"),
                       sbA.tile([P, HID], mybir.dt.float32r, name="w1k1"),
                       sbA.tile([64, HID], mybir.dt.float32r, name="w1k2")]
                nc.sync.dma_start(out=w1t[0][:], in_=w1_in[0:P, :])
                nc.sync.dma_start(out=w1t[1][:], in_=w1_in[P:2 * P, :])
                nc.sync.dma_start(out=w1t[2][:], in_=w1_in[2 * P:2 * P + 64, :])

                agg = [sbA.tile([P, npc], mybir.dt.float32r, name="agg0"),
                       sbA.tile([P, npc], mybir.dt.float32r, name="agg1"),
                       sbA.tile([64, npc], mybir.dt.float32r, name="agg2")]

                # ---- phase B: gather + aggregate + dst scale --------------
                toff = 0
                for b in range(nblk):
                    Tb = TBs[b]
                    ni = Tb * P
                    nfg = sbG.tile([P, Tb, NFS_W], mybir.dt.bfloat16, tag="nfg")
                    nc.gpsimd.dma_gather(
                        out_ap=nfg[:], in_ap=nfs_in[:],
                        idxs_ap=src16[:, toff * 8:(toff + Tb) * 8],
                        num_idxs=ni, num_idxs_reg=ni, elem_size=NFS_W,
                        single_packet=False)
                    pa = [ps1.tile([P, NB], mybir.dt.float32, space="PSUM",
                                   tag="pa0", name="pa0"),
                          ps1.tile([P, NB], mybir.dt.float32, space="PSUM",
                                   tag="pa1", name="pa1"),
                          ps1.tile([64, NB], mybir.dt.float32, space="PSUM",
                                   tag="pa2", name="pa2")]
                    for t in range(Tb):
                        s1t = s1_all[:, (toff + t) * NB:(toff + t + 1) * NB]
                        for fc, (f0, fw) in enumerate(FCH):
                            nc.tensor.matmul(
                                out=pa[fc][:, :],
                                lhsT=nfg[:, t, f0:f0 + fw], rhs=s1t,
                                start=(t == 0), stop=(t == Tb - 1))
                    dvb = ps2.tile([P, NB], mybir.dt.float32, space="PSUM",
                                   tag="dvb", bufs=1)
                    nc.tensor.matmul(out=dvb[:], lhsT=ones_row[:],
                                     rhs=dvrow[:, b * NB:(b + 1) * NB],
                                     start=True, stop=True)
                    dvbs = sbS.tile([P, NB], mybir.dt.float32, tag="dvbs")
                    nc.vector.tensor_copy(dvbs[:], dvb[:])
                    for fc, (f0, fw) in enumerate(FCH):
                        nc.vector.tensor_tensor(
                            out=agg[fc][:, b * NB:(b + 1) * NB],
                            in0=pa[fc][:, :], in1=dvbs[0:fw, :],
                            op=mybir.AluOpType.mult)
                    toff += Tb

                # ---- phase C: x1T = lrelu(W1.T @ aggF + b1) ---------------
                x1T = [sbA.tile([P, npc], mybir.dt.bfloat16, name=f"x1T{h}")
                       for h in range(HCH)]
                for h in range(HCH):
                    for nchi in range(NCH):
                        n0 = nchi * NCW
                        px = ps2.tile([P, NCW], mybir.dt.float32, space="PSUM",
                                      tag="px", bufs=2)
                        for kc, (f0, fw) in enumerate(FCH):
                            nc.tensor.matmul(
                                out=px[:],
                                lhsT=w1t[kc][:, h * P:(h + 1) * P],
                                rhs=agg[kc][:, n0:n0 + NCW],
                                start=(kc == 0), stop=(kc == 2))
                        nc.scalar.activation(
                            out=x1T[h][:, n0:n0 + NCW], in_=px[:],
                            func=mybir.ActivationFunctionType.Prelu,
                            bias=b1c[:, h:h + 1], alpha=alph[:])

                # ---- phase D: Qs = dinv * (x1T.T @ PT) --------------------
                for nchi in range(nblk):
                    n0 = nchi * NB
                    pq = ps2.tile([NB, B], mybir.dt.float32, space="PSUM",
                                  tag="pq", bufs=2)
                    for h in range(HCH):
                        nc.tensor.matmul(
                            out=pq[:], lhsT=x1T[h][:, n0:n0 + NB],
                            rhs=PT[:, h * B:(h + 1) * B],
                            start=(h == 0), stop=(h == HCH - 1))
                    qsb = sbS.tile([NB, QS_W], mybir.dt.bfloat16, tag="qsb")
                    nc.vector.memset(qsb[:, B:QS_W], 0.0)
                    nc.vector.tensor_scalar(
                        out=qsb[:, 0:B], in0=pq[:],
                        scalar1=dv2d[:, nchi:nchi + 1], scalar2=None,
                        op0=mybir.AluOpType.mult)
                    nc.sync.dma_start(out=qs_out[n0:n0 + NB, :], in_=qsb[:])
    nc.finalize()
    return nc


def _build_l3(TBs, cfg, reps=1):
    """Layer-2 aggregation over Qs -> output shard [B, npc]."""
    nc = _new_nc()
    nblk = len(TBs)
    TT = sum(TBs)
    N, B = cfg["N"], cfg["B"]
    ncores = cfg["CORES"]
    npc = N // ncores

    qs_in = nc.dram_tensor("qs", [N, QS_W], mybir.dt.bfloat16,
                           kind="ExternalInput")
    s1_in = nc.dram_tensor("s1", [P, TT * NB], mybir.dt.bfloat16,
                           kind="ExternalInput")
    src_in = nc.dram_tensor("src16", [P, TT * 8], mybir.dt.int16,
                            kind="ExternalInput")
    dvrow_in = nc.dram_tensor("dvrow", [1, npc], mybir.dt.float32,
                              kind="ExternalInput")
    cvp_in = nc.dram_tensor("cvp", [B, ncores], mybir.dt.float32,
                            kind="ExternalInput")
    out_own = nc.dram_tensor("outp", [B, npc], mybir.dt.float32,
                             kind="ExternalOutput")

    with TileContext(nc) as tc:
        with (
            tc.tile_pool(name="sbA", bufs=1) as sbA,
            tc.tile_pool(name="sbG", bufs=3) as sbG,
            tc.tile_pool(name="sbS", bufs=4) as sbS,
            tc.tile_pool(name="ps", bufs=2, space="PSUM") as ps,
        ):
            with _maybe_reps(tc, reps):
                s1_all = sbA.tile([P, TT * NB], mybir.dt.bfloat16)
                src16 = sbA.tile([P, TT * 8], mybir.dt.int16)
                dvrow = sbA.tile([1, npc], mybir.dt.float32)
                cvp = sbA.tile([B, ncores], mybir.dt.float32)
                nc.sync.dma_start(out=s1_all[:], in_=s1_in[:])
                nc.sync.dma_start(out=src16[:], in_=src_in[:])
                nc.sync.dma_start(out=dvrow[:], in_=dvrow_in[:])
                nc.sync.dma_start(out=cvp[:], in_=cvp_in[:])
                c_sb = sbA.tile([B, 1], mybir.dt.float32)
                nc.vector.tensor_reduce(out=c_sb[:], in_=cvp[:],
                                        axis=mybir.AxisListType.X,
                                        op=mybir.AluOpType.add)
                ones_row = sbA.tile([1, B], mybir.dt.float32)
                nc.vector.memset(ones_row[:], 1.0)

                toff = 0
                for b in range(nblk):
                    Tb = TBs[b]
                    ni = Tb * P
                    qg = sbG.tile([P, Tb, QS_W], mybir.dt.bfloat16, tag="qg")
                    nc.gpsimd.dma_gather(
                        out_ap=qg[:], in_ap=qs_in[:],
                        idxs_ap=src16[:, toff * 8:(toff + Tb) * 8],
                        num_idxs=ni, num_idxs_reg=ni, elem_size=QS_W,
                        single_packet=False)
                    po = ps.tile([B, NB], mybir.dt.float32, space="PSUM",
                                 tag="po")
                    for t in range(Tb):
                        s1t = s1_all[:, (toff + t) * NB:(toff + t + 1) * NB]
                        nc.tensor.matmul(out=po[:], lhsT=qg[:, t, 0:B],
                                         rhs=s1t,
                                         start=(t == 0), stop=(t == Tb - 1))
                    dvb = ps.tile([B, NB], mybir.dt.float32, space="PSUM",
                                  tag="dvb")
                    nc.tensor.matmul(out=dvb[:], lhsT=ones_row[:],
                                     rhs=dvrow[:, b * NB:(b + 1) * NB],
                                     start=True, stop=True)
                    dvbs = sbS.tile([B, NB], mybir.dt.float32, tag="dvbs")
                    nc.vector.tensor_copy(dvbs[:], dvb[:])
                    osb = sbS.tile([B, NB], mybir.dt.float32, tag="osb")
                    nc.vector.tensor_tensor(out=osb[:], in0=po[:], in1=dvbs[:],
                                            op=mybir.AluOpType.mult)
                    nc.vector.tensor_scalar(out=osb[:], in0=osb[:],
                                            scalar1=c_sb[:, 0:1], scalar2=None,
                                            op0=mybir.AluOpType.add)
                    nc.sync.dma_start(out=out_own[:, b * NB:(b + 1) * NB],
                                      in_=osb[:])
                    toff += Tb
    nc.finalize()
    return nc


# ------------------------------------------------------------------- runner

def _run(name, nc, in_maps, cores):
    kw = {}
    if TRACE:
        kw = dict(trace=True)
    res = run_bass_kernel_spmd(nc, in_maps, core_ids=list(range(cores)), **kw)
    if res.exec_time_ns is not None:
        LAST_EXEC_NS[name] = res.exec_time_ns
    return res.results


def _kernel_impl(img_feat, node_features, edge_src, edge_dst, edge_weight,
                 W1, b1, W2, b2, cfg):
    ncores = cfg["CORES"]
    N, B, HID, OUT = cfg["N"], cfg["B"], cfg["HID"], cfg["OUT"]
    npc = N // ncores
    OSL = OUT // ncores
    HCH = HID // P

    TBs, per_core = _prep_edges(edge_src, edge_dst, edge_weight, cfg)
    key = (tuple(TBs), tuple(sorted(cfg.items())))
    if key not in _BUILD_CACHE:
        _BUILD_CACHE[key] = (_build_l1(TBs, cfg), _build_l2(TBs, cfg),
                             _build_l3(TBs, cfg))
    nc1, nc2, nc3 = _BUILD_CACHE[key]

    # ---- L1
    W2T = np.ascontiguousarray(W2.T).astype(np.float32)        # [OUT, HID]
    imgT = np.ascontiguousarray(img_feat.T).astype(np.float32)  # [OUT, B]
    maps1 = [dict(dstl=pc["dstl"], wts=pc["wts"],
                  nfslab=np.ascontiguousarray(
                      node_features[k * npc:(k + 1) * npc]).astype(np.float32),
                  w2ts=np.ascontiguousarray(W2T[k * OSL:(k + 1) * OSL]),
                  imgts=np.ascontiguousarray(imgT[k * OSL:(k + 1) * OSL]),
                  b2s=np.ascontiguousarray(
                      b2[k * OSL:(k + 1) * OSL].reshape(-1, 1)).astype(
                          np.float32))
             for k, pc in enumerate(per_core)]
    r1 = _run("l1", nc1, maps1, ncores)
    nfs = np.concatenate([r1[k]["nfs"] for k in range(ncores)], axis=0)
    ptp = np.concatenate([r1[k]["ptp"] for k in range(ncores)], axis=1)
    cvp = np.concatenate([r1[k]["cvp"] for k in range(ncores)], axis=1)

    # ---- L2
    w1_pad = np.zeros((FPAD, HID), np.float32)
    w1_pad[:F_TEXT, :] = W1
    b1c = np.ascontiguousarray(
        np.asarray(b1, np.float32).reshape(HCH, P).T)           # [128, 8]
    maps2 = [dict(nfs=nfs, s1=r1[k]["s1"], src16=pc["src16"],
                  dv2d=r1[k]["dv2d"], dvrow=r1[k]["dvrow"],
                  ptp=ptp, w1=w1_pad, b1c=b1c)
             for k, pc in enumerate(per_core)]
    r2 = _run("l2", nc2, maps2, ncores)
    qs = np.concatenate([r2[k]["qs"] for k in range(ncores)], axis=0)

    # ---- L3
    maps3 = [dict(qs=qs, s1=r1[k]["s1"], src16=pc["src16"],
                  dvrow=r1[k]["dvrow"], cvp=cvp)
             for k, pc in enumerate(per_core)]
    r3 = _run("l3", nc3, maps3, ncores)
    out = np.concatenate([r3[k]["outp"] for k in range(ncores)], axis=1)

    global LAST_BUILD, LAST_MAPS, LAST_REP_BUILDERS
    LAST_BUILD = (nc1, nc2, nc3)
    LAST_MAPS = {"l1": maps1, "l2": maps2, "l3": maps3}
    LAST_REP_BUILDERS = {
        "l1": lambda reps: _build_l1(TBs, cfg, reps=reps),
        "l2": lambda reps: _build_l2(TBs, cfg, reps=reps),
        "l3": lambda reps: _build_l3(TBs, cfg, reps=reps),
    }
    return out.astype(np.float32)


def kernel(img_feat, node_features, edge_src, edge_dst, edge_weight,
           W1, b1, W2, b2):
    return _kernel_impl(np.asarray(img_feat), np.asarray(node_features),
                        np.asarray(edge_src), np.asarray(edge_dst),
                        np.asarray(edge_weight), np.asarray(W1),
                        np.asarray(b1), np.asarray(W2), np.asarray(b2),
                        CFG_FULL)
